# revision 8
# baseline (speedup 1.0000x reference)
"""Trainium2 Bass kernel for nn_Discriminator (histogram_binning / ridge).

Math (reference):
  For each batch n (N=32): interpolate P=128 points into M=(P-1)*181=22987
  line points (x,y,w); splat Gaussians g_x[m,s]=exp(-(x_m-s)^2/(2 w_m)),
  g_y[m,t]; canvas = g_x^T @ g_y  [128,128]; line = tanh(canvas);
  loss = sum(BCE(line, img))/N + sum(poly_sqrt(seg_len^2))/N.

Quadrature resampling (host):
  The 181 samples/segment wildly oversample the Gaussian (sigma=sqrt(w)
  in [0.7,1.4]).  A midpoint rule at spacing H*sigma_min along each
  segment reproduces the discrete splat sum with relative ripple
  ~2*exp(-2*pi^2*sigma^2/h^2) (theta-function); the quadrature weight
  rho=181/n folds into the Gaussian exponent as +ln(rho)/2 per axis.
  The 128 original polyline vertices are appended as explicit weight-1/2
  samples to repair the Euler-Maclaurin end/corner terms that dominate
  the deep-tail log(canvas).  H=2.8 -> ~4.2k points (33 chunks) per
  batch vs 180, measured loss rel err ~8e-4 (gate 2e-2).

Device strategy (data-parallel over N, 4 batches per core, 8 cores):
  The Gaussian exponent arg[m,s] = c2[m]*s'^2 + c1[m]*s' + c0[m] (s'=s-64)
  is computed on the TensorEngine as a K=24 bf16 matmul: the basis rows
  (s'^2 split into two exactly-representable bf16 rows, s', 1) are exact,
  and each coefficient is split into 3 bf16 levels (~25-bit mantissa).
  A block-diagonal basis computes the x-arg and y-arg in one matmul
  ([24,128] lhsT x [24,256] rhs -> [128m, 256]). ScalarE applies one Exp
  per element (PSUM->SBUF, bf16 out), and the canvas accumulates NCHUNK
  chunk matmuls (K=128, bf16) in PSUM. tanh/log/BCE epilogue per batch,
  free-dim reduced on DVE; final partition sums on host.
"""
import sys
import types
import numpy as np
import ml_dtypes

# ---------------------------------------------------------------- constants
IMG = 128          # image size S
P = 128            # points per batch
N = 32             # batch
CMP = int(IMG * np.sqrt(2))            # 181
H_SPACING = 2.8    # quadrature spacing in units of sigma_min
NCHUNK = 36        # quadrature points per batch, in chunks of 128
MPAD = NCHUNK * 128                    # 4608
NCORES = 8
NB = N // NCORES                       # 4 batches per core
GRP = 6                                # arg chunks per Exp instruction
NGRP = NCHUNK // GRP                   # 6
CENTER = 64.0

_d = np.arange(-IMG + 1, IMG)
X0 = float((_d ** 2 + (_d ** 2).T).mean().astype(np.float32))
C0 = float(X0 ** 0.5)
C1 = float(X0 ** (-0.5) / 2.0)
C2 = float(-(X0 ** (-1.5) / 8.0))
C3 = float(X0 ** (-2.5) / 16.0)

_BF = ml_dtypes.bfloat16

# XLA:CPU f32 tanh returns exactly 1.0 for x >= this (empirical, bit-exact);
# the reference's clip(log(1-line), -100) then yields -100 on those pixels.
TANH_SAT = float(np.uint32(1090516548).view(np.float32))  # 7.9988117
ULP_BELOW_1 = 5.960464477539063e-08  # 1 - nextafter(1, 0) in f32


def _install_ntff_hook():
    """bass_utils wants antenv.axon_hooks for trace=True under axon; the image
    lacks it. Provide it, backed by the ctypes shim in trn_agent_boot."""
    if 'antenv.axon_hooks' in sys.modules:
        return
    mod = types.ModuleType('antenv.axon_hooks')
    _h = [None]
    mod.set_axon_ntff_profile_hook = lambda h: _h.__setitem__(0, h)
    mod.get_axon_ntff_profile_hook = lambda: _h[0]
    sys.modules['antenv.axon_hooks'] = mod
    try:
        from trn_agent_boot.trn_boot import _ntff_profile_via_ctypes
        mod.set_axon_ntff_profile_hook(
            _ntff_profile_via_ctypes('/opt/axon/libaxon_pjrt.so'))
    except Exception:
        pass


_install_ntff_hook()

import concourse.bass as bass          # noqa: E402
import concourse.tile as tile          # noqa: E402
from concourse import bacc, mybir      # noqa: E402
from concourse.bass_utils import run_bass_kernel_spmd  # noqa: E402

dt = mybir.dt
AF = mybir.ActivationFunctionType
ALU = mybir.AluOpType


# ---------------------------------------------------------------- host prep
def _bf16_split3(x):
    h = x.astype(_BF).astype(np.float64)
    m = (x - h).astype(_BF).astype(np.float64)
    l = (x - h - m).astype(_BF).astype(np.float64)
    return h, m, l


def _build_q24():
    """Block-diagonal exact bf16 basis, K=24 rows (12 per axis)."""
    sprime = np.arange(IMG, dtype=np.float64) - CENTER
    s2 = sprime ** 2
    s2h = s2.astype(_BF).astype(np.float64)
    s2l = s2 - s2h
    qrows = [s2h, s2l, sprime, np.ones(IMG)]
    q = np.zeros((24, 2 * IMG))
    for base, off in ((0, 0), (12, IMG)):
        for lvl in range(3):
            for j in range(4):
                q[base + lvl * 4 + j, off:off + IMG] = qrows[j]
    return q.astype(_BF)


def _quad_points(p):
    """p [P,3] f64 polyline -> (lp [Mq,3] sample points, ww [Mq] weights).

    Midpoint quadrature at spacing H_SPACING*sigma_min per segment plus
    the P original vertices at weight 1/2 (end/corner repair)."""
    a, b = p[:-1], p[1:]
    L = np.hypot(b[:, 0] - a[:, 0], b[:, 1] - a[:, 1])
    sig = np.sqrt(np.minimum(a[:, 2], b[:, 2]))
    n = np.maximum(1, np.ceil(L / (H_SPACING * sig))).astype(np.int64)
    budget = MPAD - P
    while n.sum() > budget:  # graceful degrade for adversarial inputs
        f = budget / float(n.sum())
        n = np.maximum(1, (n.astype(np.float64) * f).astype(np.int64))
        if n.sum() <= len(n):
            break
    Mq = int(n.sum())
    seg = np.repeat(np.arange(len(n)), n)
    cum = np.concatenate([[0], np.cumsum(n)])
    within = np.arange(Mq) - cum[seg]
    tt = ((within + 0.5) / n[seg])[:, None]
    lp = (1.0 - tt) * a[seg] + tt * b[seg]
    ww = CMP / n[seg].astype(np.float64)
    lp = np.concatenate([lp, p])
    ww = np.concatenate([ww, np.full(len(p), 0.5)])
    return lp, ww


def _build_f24(points):
    """points [N, P, 3] float -> F [N, 24, MPAD] bf16 coefficient rows."""
    pts = np.asarray(points, np.float64)
    F = np.zeros((N, 24, MPAD))
    for nb in range(N):
        lp, ww = _quad_points(pts[nb])
        Mq = len(lp)
        x = lp[:, 0] - CENTER
        y = lp[:, 1] - CENTER
        invw = 1.0 / lp[:, 2]
        hlw = 0.5 * np.log(ww)
        c2 = -0.5 * invw
        c1x = x * invw
        c0x = -0.5 * x * x * invw + hlw
        c1y = y * invw
        c0y = -0.5 * y * y * invw + hlw
        for base, c1_, c0_ in ((0, c1x, c0x), (12, c1y, c0y)):
            splits = [_bf16_split3(c2), _bf16_split3(c2),
                      _bf16_split3(c1_), _bf16_split3(c0_)]
            for lvl in range(3):
                for j in range(4):
                    F[nb, base + lvl * 4 + j, :Mq] = splits[j][lvl]
        # padding m in [Mq, MPAD): force arg_x = arg_y = -50 -> g ~ 0
        F[nb, 3, Mq:] = -50.0
        F[nb, 15, Mq:] = -50.0
    return F.astype(_BF)


# ---------------------------------------------------------------- device
def _build_nc():
    nc = bacc.Bacc("TRN2", target_bir_lowering=False, debug=False,
                   enable_asserts=False, num_devices=NCORES)
    f_in = nc.dram_tensor("f24", [NB, 24, MPAD], dt.bfloat16,
                          kind="ExternalInput").ap()
    q_in = nc.dram_tensor("q24", [24, 2 * IMG], dt.bfloat16,
                          kind="ExternalInput").ap()
    img_in = nc.dram_tensor("img", [NB, IMG, IMG], dt.float32,
                            kind="ExternalInput").ap()
    ptsd_in = nc.dram_tensor("ptsd", [P - 1, 2 * NB], dt.float32,
                             kind="ExternalInput").ap()
    out = nc.dram_tensor("out", [128, 2 * NB], dt.float32,
                         kind="ExternalOutput").ap()

    LN2 = 0.6931471805599453

    with tile.TileContext(nc) as tc:
        with tc.tile_pool(name="const", bufs=1) as const_pool, \
             tc.tile_pool(name="fpool", bufs=2) as fpool, \
             tc.tile_pool(name="gpool", bufs=3) as gpool, \
             tc.tile_pool(name="small", bufs=2) as small, \
             tc.tile_pool(name="canv", bufs=2) as canv_pool, \
             tc.tile_pool(name="epi", bufs=2) as epi, \
             tc.tile_pool(name="argps", bufs=2, space="PSUM") as argps, \
             tc.tile_pool(name="canps", bufs=2, space="PSUM") as canps:

            qt = const_pool.tile([24, 2 * IMG], dt.bfloat16)
            nc.sync.dma_start(qt[:], q_in[:])
            outsb = const_pool.tile([128, 2 * NB], dt.float32)
            nc.vector.memset(outsb[:], 0.0)
            m100 = const_pool.tile([128, IMG], dt.float32)
            nc.vector.memset(m100[:], -100.0)
            mant_mask = const_pool.tile([128, 1], dt.int32)
            nc.vector.memset(mant_mask[:], 0x007FFFFF)
            one_bits = const_pool.tile([128, 1], dt.int32)
            nc.vector.memset(one_bits[:], 0x3F800000)

            def epilogue(n, canvas_sb):
                """BCE row-sums for batch n, exp/ln only (no tanh):
                  log1mp = ln2 - 2c - ln(1+u),  u = exp(-2c)   [c >= 0]
                  logp   = ln(1-u) - ln(1+u);  for c<0.01 use exact
                           bitfield ln(c) (= ln tanh c to 3e-5 there);
                  f32-semantics masks: c >= TANH_SAT -> log1mp = -100,
                  c < 1e-38 -> logp = -100."""
                imgt = small.tile([128, IMG], dt.float32, name="imgt")
                nc.sync.dma_start(imgt[:], img_in[n])
                c = canvas_sb[:]
                xb = c.bitcast(dt.int32)
                u = epi.tile([128, IMG], dt.float32, name="u")
                nc.scalar.activation(u[:], c, AF.Exp, scale=-2.0)
                # Ln staging: [0:128]=1+u, [128:256]=mantissa(c), [256:384]=1-u
                trip = epi.tile([128, 3 * IMG], dt.int32, name="trip")
                nc.vector.tensor_scalar(trip[:, IMG:2 * IMG], xb,
                                        mant_mask[:, 0:1], one_bits[:, 0:1],
                                        ALU.bitwise_and, ALU.bitwise_or)
                nc.vector.tensor_scalar(trip[:, 0:IMG].bitcast(dt.float32),
                                        u[:], 1.0, None, ALU.add)
                nc.vector.tensor_scalar(trip[:, 2 * IMG:].bitcast(dt.float32),
                                        u[:], -1.0, 1.0, ALU.mult, ALU.add)
                db = epi.tile([128, IMG], dt.int32, name="db")
                nc.vector.tensor_tensor(db[:], xb, trip[:, IMG:2 * IMG],
                                        ALU.subtract)
                ef = epi.tile([128, IMG], dt.float32, name="ef")
                nc.vector.tensor_copy(ef[:], db[:])
                nc.vector.tensor_scalar(ef[:], ef[:], LN2 / (1 << 23),
                                        None, ALU.mult)
                lns = epi.tile([128, 3 * IMG], dt.float32, name="lns")
                nc.scalar.activation(lns[:], trip[:].bitcast(dt.float32),
                                     AF.Ln)
                # exact ln(c) = ln(mant) + (expfield bits)*ln2/2^23
                lnc = epi.tile([128, IMG], dt.float32, name="lnc")
                nc.vector.tensor_tensor(lnc[:], lns[:, IMG:2 * IMG], ef[:],
                                        ALU.add)
                logp = epi.tile([128, IMG], dt.float32, name="logp")
                nc.vector.tensor_tensor(logp[:], lns[:, 2 * IMG:],
                                        lns[:, 0:IMG], ALU.subtract)
                maskt = epi.tile([128, IMG], dt.uint8, name="maskt")
                nc.vector.tensor_scalar(maskt[:], c, 0.01, None, ALU.is_lt)
                nc.vector.copy_predicated(logp[:], maskt[:], lnc[:])
                nc.vector.tensor_scalar(maskt[:], c, 1e-38, None, ALU.is_lt)
                nc.vector.copy_predicated(logp[:], maskt[:], m100[:])
                log1mp = epi.tile([128, IMG], dt.float32, name="log1mp")
                nc.vector.tensor_scalar(log1mp[:], c, -2.0, LN2,
                                        ALU.mult, ALU.add)
                nc.vector.tensor_tensor(log1mp[:], log1mp[:], lns[:, 0:IMG],
                                        ALU.subtract)
                nc.vector.tensor_scalar(maskt[:], c, TANH_SAT, None,
                                        ALU.is_ge)
                nc.vector.copy_predicated(log1mp[:], maskt[:], m100[:])
                diff = epi.tile([128, IMG], dt.float32, name="diff")
                nc.vector.tensor_tensor(diff[:], logp[:], log1mp[:],
                                        ALU.subtract)
                prod = epi.tile([128, IMG], dt.float32, name="prod")
                nc.vector.tensor_tensor(prod[:], imgt[:], diff[:], ALU.mult)
                tot = epi.tile([128, IMG], dt.float32, name="tot")
                nc.vector.tensor_tensor(tot[:], prod[:], log1mp[:], ALU.add)
                nc.vector.tensor_reduce(outsb[:, n:n + 1], tot[:],
                                        mybir.AxisListType.X, ALU.add)

            prev = None  # (n, canvas_sb) pending epilogue
            for n in range(NB):
                ft = fpool.tile([24, MPAD], dt.bfloat16, name="ft")
                for sl in range(2):
                    w = MPAD // 2
                    nc.sync.dma_start(ft[:, sl * w:(sl + 1) * w],
                                      f_in[n][:, sl * w:(sl + 1) * w])

                canvas_ps = canps.tile([128, IMG], dt.float32,
                                       name="canvas_ps")
                gxys = {}
                for g in range(NGRP):
                    arg_ps = argps.tile([128, GRP * 2 * IMG], dt.float32,
                                        name="arg_ps")
                    for i in range(GRP):
                        ch = g * GRP + i
                        nc.tensor.matmul(
                            arg_ps[:, i * 2 * IMG:(i + 1) * 2 * IMG],
                            ft[:, ch * 128:(ch + 1) * 128], qt[:],
                            start=True, stop=True)
                    gxy = gpool.tile([128, GRP * 2 * IMG], dt.bfloat16,
                                     name="gxy")
                    nc.scalar.activation(gxy[:], arg_ps[:], AF.Exp)
                    gxys[g] = gxy
                    # previous batch's epilogue rides between this batch's
                    # exp groups (same ACT table set: exp+ln)
                    if g == 1 and prev is not None:
                        epilogue(*prev)
                        prev = None
                    # software pipeline: canvas matmuls for group g-1 are
                    # emitted after group g's args so the PE never waits
                    # on the exp of the group it just computed
                    if g > 0:
                        for i in range(GRP):
                            ch = (g - 1) * GRP + i
                            o = i * 2 * IMG
                            gp = gxys[g - 1]
                            nc.tensor.matmul(
                                canvas_ps[:],
                                gp[:, o:o + IMG], gp[:, o + IMG:o + 2 * IMG],
                                start=(ch == 0), stop=False)
                for i in range(GRP):
                    ch = (NGRP - 1) * GRP + i
                    o = i * 2 * IMG
                    gp = gxys[NGRP - 1]
                    nc.tensor.matmul(
                        canvas_ps[:],
                        gp[:, o:o + IMG], gp[:, o + IMG:o + 2 * IMG],
                        start=False, stop=(ch == NCHUNK - 1))

                canvas_sb = canv_pool.tile([128, IMG], dt.float32,
                                           name="canvas_sb")
                nc.vector.tensor_copy(canvas_sb[:], canvas_ps[:])
                prev = (n, canvas_sb)

            epilogue(*prev)

            # ---- distance term, all NB batches at once:
            # ptsd = [127, dx(4) | dy(4)]
            pd = small.tile([P - 1, 2 * NB], dt.float32, name="pd")
            nc.sync.dma_start(pd[:], ptsd_in[:])
            sq = epi.tile([P - 1, 2 * NB], dt.float32, name="sq")
            nc.vector.tensor_tensor(sq[:], pd[:], pd[:], ALU.mult)
            dxp = epi.tile([P - 1, NB], dt.float32, name="dxp")
            nc.vector.tensor_tensor(dxp[:], sq[:, 0:NB], sq[:, NB:2 * NB],
                                    ALU.add)
            nc.vector.tensor_scalar(dxp[:], dxp[:], -X0, None, ALU.add)
            poly = epi.tile([P - 1, NB], dt.float32, name="poly")
            nc.vector.tensor_scalar(poly[:], dxp[:], C3, C2,
                                    ALU.mult, ALU.add)
            nc.vector.tensor_tensor(poly[:], poly[:], dxp[:], ALU.mult)
            nc.vector.tensor_scalar(poly[:], poly[:], C1, None, ALU.add)
            nc.vector.tensor_tensor(poly[:], poly[:], dxp[:], ALU.mult)
            nc.vector.tensor_scalar(outsb[:P - 1, NB:2 * NB], poly[:],
                                    C0, None, ALU.add)

            nc.sync.dma_start(out[:], outsb[:])
    nc.compile()
    return nc


_NC_CACHE = None


def _get_nc():
    global _NC_CACHE
    if _NC_CACHE is None:
        _NC_CACHE = _build_nc()
    return _NC_CACHE


def make_in_maps(points, img):
    points = np.asarray(points, np.float32)
    img = np.asarray(img, np.float32)
    f24 = _build_f24(points)                   # [N, 24, MPAD] bf16
    q24 = _build_q24()                         # [24, 256] bf16
    deltas = points[:, 1:, 0:2] - points[:, :-1, 0:2]   # [N, 127, 2]
    in_maps = []
    for c in range(NCORES):
        sl = slice(c * NB, (c + 1) * NB)
        # ptsd: [127, dx cols for NB batches | dy cols for NB batches]
        d = deltas[sl]                          # [NB, 127, 2]
        ptsd = np.concatenate([d[:, :, 0].T, d[:, :, 1].T], axis=1)
        in_maps.append({
            "f24": np.ascontiguousarray(f24[sl]),
            "q24": q24,
            "img": np.ascontiguousarray(img[sl]),
            "ptsd": np.ascontiguousarray(ptsd),
        })
    return in_maps


def combine_outputs(results):
    bce_tot = 0.0
    dist_tot = 0.0
    for r in results:
        o = np.asarray(r["out"], np.float64)
        bce_tot += o[:, :NB].sum()
        dist_tot += o[:P - 1, NB:].sum()
    return np.float32((dist_tot - bce_tot) / N)


def kernel(points, img, _trace=False, _trace_kwargs=None):
    nc = _get_nc()
    in_maps = make_in_maps(points, img)
    kw = {}
    if _trace:
        kw.update(trace=True, trace_cores=[0])
        if _trace_kwargs:
            kw.update(_trace_kwargs)
    res = run_bass_kernel_spmd(nc, in_maps, core_ids=list(range(NCORES)), **kw)
    out = combine_outputs(res.results)
    if _trace:
        return out, res
    return out



# revision 11
# speedup vs baseline: 1.1470x; 1.1470x over previous
"""Trainium2 Bass kernel for nn_Discriminator (histogram_binning / ridge).

Math (reference):
  For each batch n (N=32): interpolate P=128 points into M=(P-1)*181=22987
  line points (x,y,w); splat Gaussians g_x[m,s]=exp(-(x_m-s)^2/(2 w_m)),
  g_y[m,t]; canvas = g_x^T @ g_y  [128,128]; line = tanh(canvas);
  loss = sum(BCE(line, img))/N + sum(poly_sqrt(seg_len^2))/N.

Quadrature resampling (host):
  The 181 samples/segment wildly oversample the Gaussian (sigma=sqrt(w)
  in [0.7,1.4]).  A midpoint rule at spacing H*sigma_min along each
  segment reproduces the discrete splat sum with relative ripple
  ~2*exp(-2*pi^2*sigma^2/h^2) (theta-function); the quadrature weight
  rho=181/n folds into the Gaussian exponent as +ln(rho)/2 per axis.
  The 128 original polyline vertices are appended as explicit weight-1/2
  samples to repair the Euler-Maclaurin end/corner terms that dominate
  the deep-tail log(canvas).  H=2.8 -> ~4.2k points (33 chunks) per
  batch vs 180, measured loss rel err ~8e-4 (gate 2e-2).

Device strategy (data-parallel over N, 4 batches per core, 8 cores):
  The Gaussian exponent arg[m,s] = c2[m]*s'^2 + c1[m]*s' + c0[m] (s'=s-64)
  is computed on the TensorEngine as a K=24 bf16 matmul: the basis rows
  (s'^2 split into two exactly-representable bf16 rows, s', 1) are exact,
  and each coefficient is split into 3 bf16 levels (~25-bit mantissa).
  A block-diagonal basis computes the x-arg and y-arg in one matmul
  ([24,128] lhsT x [24,256] rhs -> [128m, 256]). ScalarE applies one Exp
  per element (PSUM->SBUF, bf16 out), and the canvas accumulates NCHUNK
  chunk matmuls (K=128, bf16) in PSUM. tanh/log/BCE epilogue per batch,
  free-dim reduced on DVE; final partition sums on host.
"""
import sys
import types
import numpy as np
import ml_dtypes

# ---------------------------------------------------------------- constants
IMG = 128          # image size S
P = 128            # points per batch
N = 32             # batch
CMP = int(IMG * np.sqrt(2))            # 181
H_SPACING = 2.8    # quadrature spacing in units of sigma_min
NCHUNK = 36        # quadrature points per batch, in chunks of 128
MPAD = NCHUNK * 128                    # 4608
NCORES = 8
NB = N // NCORES                       # 4 batches per core
GRP = 6                                # arg chunks per Exp instruction
NGRP = NCHUNK // GRP                   # 6
CENTER = 64.0

_d = np.arange(-IMG + 1, IMG)
X0 = float((_d ** 2 + (_d ** 2).T).mean().astype(np.float32))
C0 = float(X0 ** 0.5)
C1 = float(X0 ** (-0.5) / 2.0)
C2 = float(-(X0 ** (-1.5) / 8.0))
C3 = float(X0 ** (-2.5) / 16.0)

_BF = ml_dtypes.bfloat16

# XLA:CPU f32 tanh returns exactly 1.0 for x >= this (empirical, bit-exact);
# the reference's clip(log(1-line), -100) then yields -100 on those pixels.
TANH_SAT = float(np.uint32(1090516548).view(np.float32))  # 7.9988117
ULP_BELOW_1 = 5.960464477539063e-08  # 1 - nextafter(1, 0) in f32


def _install_ntff_hook():
    """bass_utils wants antenv.axon_hooks for trace=True under axon; the image
    lacks it. Provide it, backed by the ctypes shim in trn_agent_boot."""
    if 'antenv.axon_hooks' in sys.modules:
        return
    mod = types.ModuleType('antenv.axon_hooks')
    _h = [None]
    mod.set_axon_ntff_profile_hook = lambda h: _h.__setitem__(0, h)
    mod.get_axon_ntff_profile_hook = lambda: _h[0]
    sys.modules['antenv.axon_hooks'] = mod
    try:
        from trn_agent_boot.trn_boot import _ntff_profile_via_ctypes
        mod.set_axon_ntff_profile_hook(
            _ntff_profile_via_ctypes('/opt/axon/libaxon_pjrt.so'))
    except Exception:
        pass


_install_ntff_hook()

import concourse.bass as bass          # noqa: E402
import concourse.tile as tile          # noqa: E402
from concourse import bacc, mybir      # noqa: E402
from concourse.bass_utils import run_bass_kernel_spmd  # noqa: E402

dt = mybir.dt
AF = mybir.ActivationFunctionType
ALU = mybir.AluOpType


# ---------------------------------------------------------------- host prep
def _bf16_split3(x):
    h = x.astype(_BF).astype(np.float64)
    m = (x - h).astype(_BF).astype(np.float64)
    l = (x - h - m).astype(_BF).astype(np.float64)
    return h, m, l


def _build_q24():
    """Block-diagonal exact bf16 basis, zero-padded to K=128 rows (the PE's
    HAM clock-gate only counts full-K matmuls as activity; K=24 matmuls
    down-clock the PE to 1.2 GHz — measured 1.4x slowdown)."""
    sprime = np.arange(IMG, dtype=np.float64) - CENTER
    s2 = sprime ** 2
    s2h = s2.astype(_BF).astype(np.float64)
    s2l = s2 - s2h
    qrows = [s2h, s2l, sprime, np.ones(IMG)]
    q = np.zeros((128, 2 * IMG))
    for base, off in ((0, 0), (12, IMG)):
        for lvl in range(3):
            for j in range(4):
                q[base + lvl * 4 + j, off:off + IMG] = qrows[j]
    return q.astype(_BF)


def _quad_points(p):
    """p [P,3] f64 polyline -> (lp [Mq,3] sample points, ww [Mq] weights).

    Midpoint quadrature at spacing H_SPACING*sigma_min per segment plus
    the P original vertices at weight 1/2 (end/corner repair)."""
    a, b = p[:-1], p[1:]
    L = np.hypot(b[:, 0] - a[:, 0], b[:, 1] - a[:, 1])
    sig = np.sqrt(np.minimum(a[:, 2], b[:, 2]))
    n = np.maximum(1, np.ceil(L / (H_SPACING * sig))).astype(np.int64)
    budget = MPAD - P
    while n.sum() > budget:  # graceful degrade for adversarial inputs
        f = budget / float(n.sum())
        n = np.maximum(1, (n.astype(np.float64) * f).astype(np.int64))
        if n.sum() <= len(n):
            break
    Mq = int(n.sum())
    seg = np.repeat(np.arange(len(n)), n)
    cum = np.concatenate([[0], np.cumsum(n)])
    within = np.arange(Mq) - cum[seg]
    tt = ((within + 0.5) / n[seg])[:, None]
    lp = (1.0 - tt) * a[seg] + tt * b[seg]
    ww = CMP / n[seg].astype(np.float64)
    lp = np.concatenate([lp, p])
    ww = np.concatenate([ww, np.full(len(p), 0.5)])
    return lp, ww


def _build_f24(points):
    """points [N, P, 3] float -> F [N, 24, MPAD] bf16 coefficient rows."""
    pts = np.asarray(points, np.float64)
    F = np.zeros((N, 24, MPAD))
    for nb in range(N):
        lp, ww = _quad_points(pts[nb])
        Mq = len(lp)
        x = lp[:, 0] - CENTER
        y = lp[:, 1] - CENTER
        invw = 1.0 / lp[:, 2]
        hlw = 0.5 * np.log(ww)
        c2 = -0.5 * invw
        c1x = x * invw
        c0x = -0.5 * x * x * invw + hlw
        c1y = y * invw
        c0y = -0.5 * y * y * invw + hlw
        for base, c1_, c0_ in ((0, c1x, c0x), (12, c1y, c0y)):
            splits = [_bf16_split3(c2), _bf16_split3(c2),
                      _bf16_split3(c1_), _bf16_split3(c0_)]
            for lvl in range(3):
                for j in range(4):
                    F[nb, base + lvl * 4 + j, :Mq] = splits[j][lvl]
        # padding m in [Mq, MPAD): force arg_x = arg_y = -50 -> g ~ 0
        F[nb, 3, Mq:] = -50.0
        F[nb, 15, Mq:] = -50.0
    return F.astype(_BF)


# ---------------------------------------------------------------- device
def _build_nc():
    nc = bacc.Bacc("TRN2", target_bir_lowering=False, debug=False,
                   enable_asserts=False, num_devices=NCORES)
    f_in = nc.dram_tensor("f24", [NB, 24, MPAD], dt.bfloat16,
                          kind="ExternalInput").ap()
    q_in = nc.dram_tensor("q24", [128, 2 * IMG], dt.bfloat16,
                          kind="ExternalInput").ap()
    img_in = nc.dram_tensor("img", [NB, IMG, IMG], dt.float32,
                            kind="ExternalInput").ap()
    ptsd_in = nc.dram_tensor("ptsd", [P - 1, 2 * NB], dt.float32,
                             kind="ExternalInput").ap()
    out = nc.dram_tensor("out", [128, 2 * NB], dt.float32,
                         kind="ExternalOutput").ap()

    LN2 = 0.6931471805599453

    with tile.TileContext(nc) as tc:
        with tc.tile_pool(name="const", bufs=1) as const_pool, \
             tc.tile_pool(name="fpool", bufs=2) as fpool, \
             tc.tile_pool(name="gpool", bufs=3) as gpool, \
             tc.tile_pool(name="small", bufs=2) as small, \
             tc.tile_pool(name="canv", bufs=2) as canv_pool, \
             tc.tile_pool(name="epi", bufs=2) as epi, \
             tc.tile_pool(name="argps", bufs=2, space="PSUM") as argps, \
             tc.tile_pool(name="canps", bufs=2, space="PSUM") as canps:

            qt = const_pool.tile([128, 2 * IMG], dt.bfloat16)
            nc.sync.dma_start(qt[:], q_in[:])
            outsb = const_pool.tile([128, 2 * NB], dt.float32)
            nc.vector.memset(outsb[:], 0.0)
            m100 = const_pool.tile([128, IMG], dt.float32)
            nc.vector.memset(m100[:], -100.0)
            mant_mask = const_pool.tile([128, 1], dt.int32)
            nc.vector.memset(mant_mask[:], 0x007FFFFF)
            one_bits = const_pool.tile([128, 1], dt.int32)
            nc.vector.memset(one_bits[:], 0x3F800000)
            # ft double buffer: rows 24..127 stay zero forever (the PE's
            # HAM clock-gate wants full-K matmuls); DMA only rows 0..23.
            fts = []
            for b in range(2):
                ftb = const_pool.tile([128, MPAD], dt.bfloat16,
                                      name=f"ft{b}")
                nc.vector.memset(ftb[:], 0.0)
                fts.append(ftb)

            state = {}  # per-batch tiles carried from pre- to post-epilogue

            def epilogue_pre(n, canvas_sb):
                """exp-set part of the BCE epilogue for batch n:
                  u = exp(-2c); Ln staging tile [1+u | mantissa(c) | 1-u];
                  exponent-field term ef for the exact bitfield ln(c)."""
                imgt = small.tile([128, IMG], dt.float32, name="imgt",
                                  bufs=NB)
                nc.sync.dma_start(imgt[:], img_in[n])
                c = canvas_sb[:]
                xb = c.bitcast(dt.int32)
                u = epi.tile([128, IMG], dt.float32, name="u")
                nc.scalar.activation(u[:], c, AF.Exp, scale=-2.0)
                trip = epi.tile([128, 3 * IMG], dt.int32, name="trip",
                                bufs=NB)
                nc.vector.tensor_scalar(trip[:, IMG:2 * IMG], xb,
                                        mant_mask[:, 0:1], one_bits[:, 0:1],
                                        ALU.bitwise_and, ALU.bitwise_or)
                nc.vector.tensor_scalar(trip[:, 0:IMG].bitcast(dt.float32),
                                        u[:], 1.0, None, ALU.add)
                nc.vector.tensor_scalar(trip[:, 2 * IMG:].bitcast(dt.float32),
                                        u[:], -1.0, 1.0, ALU.mult, ALU.add)
                db = epi.tile([128, IMG], dt.int32, name="db")
                nc.vector.tensor_tensor(db[:], xb, trip[:, IMG:2 * IMG],
                                        ALU.subtract)
                ef = epi.tile([128, IMG], dt.float32, name="ef", bufs=NB)
                nc.vector.tensor_copy(ef[:], db[:])
                nc.vector.tensor_scalar(ef[:], ef[:], LN2 / (1 << 23),
                                        None, ALU.mult)
                state[n] = (canvas_sb, imgt, trip, ef)

            def epilogue_post(n):
                """ln-set part (one ACT table switch for all batches):
                  log1mp = ln2 - 2c - ln(1+u);  logp = ln(1-u) - ln(1+u),
                  for c<0.01 the exact bitfield ln(c);  f32-semantics
                  masks: c >= TANH_SAT -> log1mp=-100; c<1e-38 -> logp=-100."""
                canvas_sb, imgt, trip, ef = state[n]
                c = canvas_sb[:]
                lns = epi.tile([128, 3 * IMG], dt.float32, name="lns")
                nc.scalar.activation(lns[:], trip[:].bitcast(dt.float32),
                                     AF.Ln)
                lnc = epi.tile([128, IMG], dt.float32, name="lnc")
                nc.vector.tensor_tensor(lnc[:], lns[:, IMG:2 * IMG], ef[:],
                                        ALU.add)
                logp = epi.tile([128, IMG], dt.float32, name="logp")
                nc.vector.tensor_tensor(logp[:], lns[:, 2 * IMG:],
                                        lns[:, 0:IMG], ALU.subtract)
                maskt = epi.tile([128, IMG], dt.uint8, name="maskt")
                nc.vector.tensor_scalar(maskt[:], c, 0.01, None, ALU.is_lt)
                nc.vector.copy_predicated(logp[:], maskt[:], lnc[:])
                nc.vector.tensor_scalar(maskt[:], c, 1e-38, None, ALU.is_lt)
                nc.vector.copy_predicated(logp[:], maskt[:], m100[:])
                log1mp = epi.tile([128, IMG], dt.float32, name="log1mp")
                nc.vector.tensor_scalar(log1mp[:], c, -2.0, LN2,
                                        ALU.mult, ALU.add)
                nc.vector.tensor_tensor(log1mp[:], log1mp[:], lns[:, 0:IMG],
                                        ALU.subtract)
                nc.vector.tensor_scalar(maskt[:], c, TANH_SAT, None,
                                        ALU.is_ge)
                nc.vector.copy_predicated(log1mp[:], maskt[:], m100[:])
                diff = epi.tile([128, IMG], dt.float32, name="diff")
                nc.vector.tensor_tensor(diff[:], logp[:], log1mp[:],
                                        ALU.subtract)
                prod = epi.tile([128, IMG], dt.float32, name="prod")
                nc.vector.tensor_tensor(prod[:], imgt[:], diff[:], ALU.mult)
                tot = epi.tile([128, IMG], dt.float32, name="tot")
                nc.vector.tensor_tensor(tot[:], prod[:], log1mp[:], ALU.add)
                nc.vector.tensor_reduce(outsb[:, n:n + 1], tot[:],
                                        mybir.AxisListType.X, ALU.add)

            for n in range(NB):
                ft = fts[n % 2]
                for sl in range(2):
                    w = MPAD // 2
                    nc.sync.dma_start(ft[0:24, sl * w:(sl + 1) * w],
                                      f_in[n][:, sl * w:(sl + 1) * w])

                canvas_ps = canps.tile([128, IMG], dt.float32,
                                       name="canvas_ps")
                gxys = {}
                for g in range(NGRP):
                    arg_ps = argps.tile([128, GRP * 2 * IMG], dt.float32,
                                        name="arg_ps")
                    for i in range(GRP):
                        ch = g * GRP + i
                        nc.tensor.matmul(
                            arg_ps[:, i * 2 * IMG:(i + 1) * 2 * IMG],
                            ft[:, ch * 128:(ch + 1) * 128], qt[:],
                            start=True, stop=True)
                    gxy = gpool.tile([128, GRP * 2 * IMG], dt.bfloat16,
                                     name="gxy")
                    nc.scalar.activation(gxy[:], arg_ps[:], AF.Exp)
                    gxys[g] = gxy
                    # software pipeline: canvas matmuls for group g-1 are
                    # emitted after group g's args so the PE never waits
                    # on the exp of the group it just computed
                    if g > 0:
                        for i in range(GRP):
                            ch = (g - 1) * GRP + i
                            o = i * 2 * IMG
                            gp = gxys[g - 1]
                            nc.tensor.matmul(
                                canvas_ps[:],
                                gp[:, o:o + IMG], gp[:, o + IMG:o + 2 * IMG],
                                start=(ch == 0), stop=False)
                for i in range(GRP):
                    ch = (NGRP - 1) * GRP + i
                    o = i * 2 * IMG
                    gp = gxys[NGRP - 1]
                    nc.tensor.matmul(
                        canvas_ps[:],
                        gp[:, o:o + IMG], gp[:, o + IMG:o + 2 * IMG],
                        start=False, stop=(ch == NCHUNK - 1))

                canvas_sb = canv_pool.tile([128, IMG], dt.float32,
                                           name="canvas_sb", bufs=NB)
                nc.vector.tensor_copy(canvas_sb[:], canvas_ps[:])
                epilogue_pre(n, canvas_sb)

            for n in range(NB):
                epilogue_post(n)

            # ---- distance term, all NB batches at once:
            # ptsd = [127, dx(4) | dy(4)]
            pd = small.tile([P - 1, 2 * NB], dt.float32, name="pd")
            nc.sync.dma_start(pd[:], ptsd_in[:])
            sq = epi.tile([P - 1, 2 * NB], dt.float32, name="sq")
            nc.vector.tensor_tensor(sq[:], pd[:], pd[:], ALU.mult)
            dxp = epi.tile([P - 1, NB], dt.float32, name="dxp")
            nc.vector.tensor_tensor(dxp[:], sq[:, 0:NB], sq[:, NB:2 * NB],
                                    ALU.add)
            nc.vector.tensor_scalar(dxp[:], dxp[:], -X0, None, ALU.add)
            poly = epi.tile([P - 1, NB], dt.float32, name="poly")
            nc.vector.tensor_scalar(poly[:], dxp[:], C3, C2,
                                    ALU.mult, ALU.add)
            nc.vector.tensor_tensor(poly[:], poly[:], dxp[:], ALU.mult)
            nc.vector.tensor_scalar(poly[:], poly[:], C1, None, ALU.add)
            nc.vector.tensor_tensor(poly[:], poly[:], dxp[:], ALU.mult)
            nc.vector.tensor_scalar(outsb[:P - 1, NB:2 * NB], poly[:],
                                    C0, None, ALU.add)

            nc.sync.dma_start(out[:], outsb[:])
    nc.compile()
    return nc


_NC_CACHE = None


def _get_nc():
    global _NC_CACHE
    if _NC_CACHE is None:
        _NC_CACHE = _build_nc()
    return _NC_CACHE


def make_in_maps(points, img):
    points = np.asarray(points, np.float32)
    img = np.asarray(img, np.float32)
    f24 = _build_f24(points)                   # [N, 24, MPAD] bf16
    q24 = _build_q24()                         # [24, 256] bf16
    deltas = points[:, 1:, 0:2] - points[:, :-1, 0:2]   # [N, 127, 2]
    in_maps = []
    for c in range(NCORES):
        sl = slice(c * NB, (c + 1) * NB)
        # ptsd: [127, dx cols for NB batches | dy cols for NB batches]
        d = deltas[sl]                          # [NB, 127, 2]
        ptsd = np.concatenate([d[:, :, 0].T, d[:, :, 1].T], axis=1)
        in_maps.append({
            "f24": np.ascontiguousarray(f24[sl]),
            "q24": q24,
            "img": np.ascontiguousarray(img[sl]),
            "ptsd": np.ascontiguousarray(ptsd),
        })
    return in_maps


def combine_outputs(results):
    bce_tot = 0.0
    dist_tot = 0.0
    for r in results:
        o = np.asarray(r["out"], np.float64)
        bce_tot += o[:, :NB].sum()
        dist_tot += o[:P - 1, NB:].sum()
    return np.float32((dist_tot - bce_tot) / N)


def kernel(points, img, _trace=False, _trace_kwargs=None):
    nc = _get_nc()
    in_maps = make_in_maps(points, img)
    kw = {}
    if _trace:
        kw.update(trace=True, trace_cores=[0])
        if _trace_kwargs:
            kw.update(_trace_kwargs)
    res = run_bass_kernel_spmd(nc, in_maps, core_ids=list(range(NCORES)), **kw)
    out = combine_outputs(res.results)
    if _trace:
        return out, res
    return out

